# revision 1
# baseline (speedup 1.0000x reference)
"""Local (windowed) attention kernel for TRN2, 8 NeuronCores, SPMD.

Reference computation (B=4, N=8192, DIM=1024, H=16, DH=64, W=128):
    q = x @ wq ; k,v = split(x @ wkv)
    per (batch, head, window of 128): attend to [prev window, cur window]
    with causal mask (j > i + W masked), softmax, out = attn @ v
    out = out @ wo + bo

Sharding: sequence dim split into 8 contiguous chunks of 1024 tokens, one
per core.  Each core receives its x slice with a 128-token halo in front
(zeros for core 0 — matches the reference's zero-pad of the first window)
and computes q/k/v projections, attention, and the output projection for
its own tokens only.  Weights are replicated.  No collectives.

On-chip layout is feature-major (x fed pre-transposed as [dim, token]),
so every matmul uses natural HBM weight layouts and no on-chip
transposes are needed except the attention-probs transpose (done on PE).
All matmuls run in bf16 (1 cycle/row on the PE) with fp32 PSUM
accumulation; softmax runs in fp32.  exp() is computed without
max-subtraction: scores are O(1) here (|s| < ~4), so overflow is
impossible and this matches jax.nn.softmax to fp32 roundoff.
"""

import numpy as np
import ml_dtypes

import concourse.bass as bass
import concourse.bacc as bacc
import concourse.mybir as mybir
import concourse.tile as tile
from concourse.bass_utils import run_bass_kernel_spmd

B, N, DIM = 4, 8192, 1024
H, DH, W = 16, 64, 128
NCORES = 8
TOW = N // NCORES          # own tokens per core per batch   = 1024
TH = TOW + W               # with front halo                 = 1152
NW = TOW // W              # own windows per core-batch      = 8
KT = DIM // 128            # contraction tiles               = 8
MT = DIM // 128            # inner/output tiles              = 8
SCALE = DH ** -0.5

BF16 = mybir.dt.bfloat16
F32 = mybir.dt.float32
AX = mybir.AxisListType
AF = mybir.ActivationFunctionType

TRACE = False              # set by test.py to collect an NTFF profile
TRACE_KW = {}
LAST_RESULT = None         # BassKernelResults stash when TRACE
REPEAT = 1                 # whole-computation repeats inside the NEFF (bench)


def _build_bass():
    nc = bacc.Bacc(None, target_bir_lowering=False)
    xT = nc.declare_dram_parameter("xT", [B, DIM, TH], BF16, isOutput=False)
    wq = nc.declare_dram_parameter("wq", [DIM, DIM], BF16, isOutput=False)
    wkv = nc.declare_dram_parameter("wkv", [DIM, 2 * DIM], BF16, isOutput=False)
    wo = nc.declare_dram_parameter("wo", [DIM, DIM], BF16, isOutput=False)
    bo_pm = nc.declare_dram_parameter("bo_pm", [128, MT], F32, isOutput=False)
    maskT = nc.declare_dram_parameter("maskT", [128, 128], BF16, isOutput=False)
    ident = nc.declare_dram_parameter("ident", [128, 128], BF16, isOutput=False)
    outT = nc.declare_dram_parameter("outT", [B, DIM, TOW], F32, isOutput=True)

    with tile.TileContext(nc) as tc:
        with (
            tc.tile_pool(name="wpool", bufs=1) as wpool,
            tc.tile_pool(name="xpool", bufs=2) as xpool,
            tc.tile_pool(name="actpool", bufs=1) as actpool,
            tc.tile_pool(name="spool", bufs=2) as spool,
            tc.tile_pool(name="opool", bufs=3) as opool,
            tc.tile_pool(name="pscores", bufs=1, space="PSUM") as pscores,
            tc.tile_pool(name="pattnT", bufs=1, space="PSUM") as pattnT,
            tc.tile_pool(name="ppv", bufs=1, space="PSUM") as ppv,
            tc.tile_pool(name="pproj", bufs=2, space="PSUM") as pproj,
        ):
            # ---- replicated constants (loaded once) ----
            wq_sb = wpool.tile([128, KT, DIM], BF16)
            wkv_sb = wpool.tile([128, KT, 2 * DIM], BF16)
            wo_sb = wpool.tile([128, KT, DIM], BF16)
            bo_sb = wpool.tile([128, MT], F32)
            mask_sb = wpool.tile([128, 128], BF16)
            id_sb = wpool.tile([128, 128], BF16)
            for k in range(KT):
                nc.sync.dma_start(out=wq_sb[:, k, :], in_=wq[k * 128:(k + 1) * 128, :])
                nc.sync.dma_start(out=wkv_sb[:, k, :], in_=wkv[k * 128:(k + 1) * 128, :])
                nc.sync.dma_start(out=wo_sb[:, k, :], in_=wo[k * 128:(k + 1) * 128, :])
            nc.sync.dma_start(out=bo_sb, in_=bo_pm[:])
            nc.sync.dma_start(out=mask_sb, in_=maskT[:])
            nc.sync.dma_start(out=id_sb, in_=ident[:])

            for b in [bb % B for bb in range(B * REPEAT)]:
                # ---- load xT slice (feature-major, with halo) ----
                x_sb = xpool.tile([128, KT, TH], BF16, tag="x")
                nc.gpsimd.dma_start(
                    out=x_sb[:],
                    in_=xT[b].rearrange("(k p) t -> p k t", p=128),
                )

                qT = actpool.tile([128, MT, TOW], BF16, tag="qT")
                kTt = actpool.tile([128, MT, TH], BF16, tag="kT")
                v_sb = actpool.tile([128, NW + 1, DIM], BF16, tag="v")
                aoT = actpool.tile([128, MT, TOW], BF16, tag="aoT")

                # ---- q projection, feature-major: qT[m] = wq[:,m].T @ x ----
                for m in range(MT):
                    for c in range(2):
                        ps = pproj.tile([128, 512], F32, tag="proj")
                        for k in range(KT):
                            nc.tensor.matmul(
                                ps,
                                lhsT=wq_sb[:, k, m * 128:(m + 1) * 128],
                                rhs=x_sb[:, k, W + c * 512:W + (c + 1) * 512],
                                start=(k == 0),
                                stop=(k == KT - 1),
                            )
                        nc.vector.tensor_copy(
                            out=qT[:, m, c * 512:(c + 1) * 512], in_=ps
                        )

                # ---- k projection, feature-major (incl. halo) ----
                for m in range(MT):
                    for c in range(3):
                        ps = pproj.tile([128, 384], F32, tag="proj")
                        for k in range(KT):
                            nc.tensor.matmul(
                                ps,
                                lhsT=wkv_sb[:, k, m * 128:(m + 1) * 128],
                                rhs=x_sb[:, k, c * 384:(c + 1) * 384],
                                start=(k == 0),
                                stop=(k == KT - 1),
                            )
                        nc.vector.tensor_copy(
                            out=kTt[:, m, c * 384:(c + 1) * 384], in_=ps
                        )

                # ---- v projection, token-major (incl. halo) ----
                for wi in range(NW + 1):
                    for c in range(2):
                        ps = pproj.tile([128, 512], F32, tag="proj")
                        for k in range(KT):
                            nc.tensor.matmul(
                                ps,
                                lhsT=x_sb[:, k, wi * 128:(wi + 1) * 128],
                                rhs=wkv_sb[:, k, DIM + c * 512:DIM + (c + 1) * 512],
                                start=(k == 0),
                                stop=(k == KT - 1),
                            )
                        nc.vector.tensor_copy(
                            out=v_sb[:, wi, c * 512:(c + 1) * 512], in_=ps
                        )

                # ---- attention: 8 windows x 4 groups of 4 heads ----
                for w in range(NW):
                    for g in range(4):
                        sc = pscores.tile([128, 4, 512], F32, tag="scores")
                        for hh in range(4):
                            h = 4 * g + hh
                            m, r = h // 2, (h % 2) * 64
                            nc.tensor.matmul(
                                sc[:, hh, 0:2 * W],
                                lhsT=qT[r:r + 64, m, w * W:(w + 1) * W],
                                rhs=kTt[r:r + 64, m, w * W:w * W + 2 * W],
                                start=True,
                                stop=True,
                            )
                            # additive causal mask on the current-window half:
                            # sc[:, hh, W+jc] += maskT[jc, i]  (-1e30 where jc > i)
                            nc.tensor.matmul(
                                sc[:, hh, W:2 * W],
                                lhsT=mask_sb,
                                rhs=id_sb,
                                start=False,
                                stop=False,
                                skip_group_check=True,
                            )
                        # softmax (no max-subtraction; scores are O(1));
                        # exp + per-head row-sum fused on ACT via accum_out
                        exps = spool.tile([128, 4, 2 * W], F32, tag="expS")
                        sums = spool.tile([128, 4], F32, tag="sums")
                        for hh in range(4):
                            nc.scalar.activation(
                                out=exps[:, hh, :],
                                in_=sc[:, hh, 0:2 * W],
                                func=AF.Exp,
                                bias=0.0,
                                scale=SCALE,
                                accum_out=sums[:, hh:hh + 1],
                            )
                        recip = spool.tile([128, 4], F32, tag="recip")
                        nc.vector.reciprocal(out=recip, in_=sums)
                        attn = spool.tile([128, 4, 2 * W], BF16, tag="attnb")
                        for hh in range(4):
                            nc.scalar.activation(
                                out=attn[:, hh, :],
                                in_=exps[:, hh, :],
                                func=AF.Copy,
                                bias=0.0,
                                scale=recip[:, hh:hh + 1],
                            )
                        # transpose probs on PE: [i, j] -> [j, i]
                        pT = pattnT.tile([128, 8, 128], BF16, tag="attnT")
                        for hh in range(4):
                            for hf in range(2):
                                nc.tensor.transpose(
                                    out=pT[:, hh * 2 + hf, :],
                                    in_=attn[:, hh, hf * W:(hf + 1) * W],
                                    identity=id_sb,
                                )
                        aT = spool.tile([128, 8, 128], BF16, tag="attnT_sb")
                        nc.vector.tensor_copy(out=aT, in_=pT)
                        # pv: outT[dh, i] = v[j, dh].T @ attnT[j, i]
                        pv = ppv.tile([128, 2, 128], F32, tag="pv")
                        for hh in range(4):
                            h = 4 * g + hh
                            pr, pc = (hh % 2) * 64, hh // 2
                            for hf in range(2):
                                nc.tensor.matmul(
                                    pv[pr:pr + 64, pc, :],
                                    lhsT=v_sb[:, w + hf, h * 64:(h + 1) * 64],
                                    rhs=aT[:, hh * 2 + hf, :],
                                    start=(hf == 0),
                                    stop=(hf == 1),
                                )
                        for pc in range(2):
                            nc.vector.tensor_copy(
                                out=aoT[:, 2 * g + pc, w * W:(w + 1) * W],
                                in_=pv[:, pc, :],
                            )

                # ---- output projection + bias, feature-major ----
                for m in range(MT):
                    for c in range(2):
                        ps = pproj.tile([128, 512], F32, tag="proj")
                        for k in range(KT):
                            nc.tensor.matmul(
                                ps,
                                lhsT=wo_sb[:, k, m * 128:(m + 1) * 128],
                                rhs=aoT[:, k, c * 512:(c + 1) * 512],
                                start=(k == 0),
                                stop=(k == KT - 1),
                            )
                        osb = opool.tile([128, 512], F32, tag="outsb")
                        nc.vector.tensor_scalar_add(
                            out=osb, in0=ps, scalar1=bo_sb[:, m:m + 1]
                        )
                        nc.sync.dma_start(
                            out=outT[b, m * 128:(m + 1) * 128, c * 512:(c + 1) * 512],
                            in_=osb,
                        )
    nc.compile()
    return nc


_NC_CACHE = None


def _get_nc():
    global _NC_CACHE
    if _NC_CACHE is None:
        _NC_CACHE = _build_bass()
    return _NC_CACHE


def kernel(x, wq, wkv, wo, bo):
    global LAST_RESULT
    bfd = ml_dtypes.bfloat16
    x = np.asarray(x, np.float32)
    wq_b = np.asarray(wq, np.float32).astype(bfd)
    wkv_b = np.asarray(wkv, np.float32).astype(bfd)
    wo_b = np.asarray(wo, np.float32).astype(bfd)
    bo_pm = np.ascontiguousarray(
        np.asarray(bo, np.float32).reshape(MT, 128).T
    )
    # maskT[jc, i] = -1e30 where current-window col jc > row i (causal)
    maskT = np.where(
        np.arange(W)[:, None] > np.arange(W)[None, :], -1e30, 0.0
    ).astype(bfd)
    ident = np.eye(128, dtype=bfd)

    xb = x.astype(bfd)
    in_maps = []
    for c in range(NCORES):
        lo, hi = c * TOW - W, (c + 1) * TOW
        if c == 0:
            sl = np.concatenate(
                [np.zeros((B, W, DIM), bfd), xb[:, :hi]], axis=1
            )
        else:
            sl = xb[:, lo:hi]
        xT_c = np.ascontiguousarray(sl.transpose(0, 2, 1))  # [B, DIM, TH]
        in_maps.append(
            dict(xT=xT_c, wq=wq_b, wkv=wkv_b, wo=wo_b, bo_pm=bo_pm,
                 maskT=maskT, ident=ident)
        )

    nc = _get_nc()
    res = run_bass_kernel_spmd(
        nc, in_maps, list(range(NCORES)), trace=TRACE, **TRACE_KW
    )
    if TRACE:
        LAST_RESULT = res
    out = np.empty((B, N, DIM), np.float32)
    for c in range(NCORES):
        out[:, c * TOW:(c + 1) * TOW, :] = res.results[c]["outT"].transpose(0, 2, 1)
    return out



# revision 3
# speedup vs baseline: 1.0501x; 1.0501x over previous
"""Local (windowed) attention kernel for TRN2, 8 NeuronCores, SPMD.

Reference computation (B=4, N=8192, DIM=1024, H=16, DH=64, W=128):
    q = x @ wq ; k,v = split(x @ wkv)
    per (batch, head, window of 128): attend to [prev window, cur window]
    with causal mask (j > i + W masked), softmax, out = attn @ v
    out = out @ wo + bo

Sharding: sequence dim split into 8 contiguous chunks of 1024 tokens, one
per core.  Each core receives its x slice with a 128-token halo in front
(zeros for core 0 — matches the reference's zero-pad of the first window)
and computes q/k/v projections, attention, and the output projection for
its own tokens only.  Weights are replicated.  No collectives.

Processing is software-pipelined at half-batch granularity (512 own
tokens + 128-token halo per step): all activation tiles are
double-buffered so the Tile scheduler can overlap step s+1's dense
projection matmuls with step s's attention (whose PE work is interleaved
with ACT softmax / DVE copies).  This keeps the PE free of multi-us idle
gaps so the HAM clock gate stays at full rate.

On-chip layout is feature-major (x fed pre-transposed as [dim, token]),
so every matmul uses natural HBM weight layouts and no on-chip
transposes are needed except the attention-probs transpose (done on PE).
All matmuls run in bf16 (1 cycle/row on the PE) with fp32 PSUM
accumulation; softmax runs in fp32.  exp() is computed without
max-subtraction: scores are O(1) here (|s| < ~4), so overflow is
impossible and this matches jax.nn.softmax to fp32 roundoff.
"""

import numpy as np
import ml_dtypes

import concourse.bass as bass
import concourse.bacc as bacc
import concourse.mybir as mybir
import concourse.tile as tile
from concourse.bass_utils import run_bass_kernel_spmd

B, N, DIM = 4, 8192, 1024
H, DH, W = 16, 64, 128
NCORES = 8
TOW = N // NCORES          # own tokens per core per batch   = 1024
TH = TOW + W               # with front halo                 = 1152
HB = TOW // 2              # own tokens per half-batch step  = 512
HTH = HB + W               # step tokens with halo           = 640
NWH = HB // W              # windows per step                = 4
NSTEP = 2 * B              # pipeline steps per core         = 8
KT = DIM // 128            # contraction tiles               = 8
MT = DIM // 128            # inner/output tiles              = 8
SCALE = DH ** -0.5

BF16 = mybir.dt.bfloat16
F32 = mybir.dt.float32
AX = mybir.AxisListType
AF = mybir.ActivationFunctionType

TRACE = False              # set by test.py to collect an NTFF profile
TRACE_KW = {}
LAST_RESULT = None         # BassKernelResults stash when TRACE
REPEAT = 1                 # whole-computation repeats inside the NEFF (bench)


def _build_bass():
    nc = bacc.Bacc(None, target_bir_lowering=False)
    xT = nc.declare_dram_parameter("xT", [B, DIM, TH], BF16, isOutput=False)
    wq = nc.declare_dram_parameter("wq", [DIM, DIM], BF16, isOutput=False)
    wkv = nc.declare_dram_parameter("wkv", [DIM, 2 * DIM], BF16, isOutput=False)
    wo = nc.declare_dram_parameter("wo", [DIM, DIM], BF16, isOutput=False)
    bo_pm = nc.declare_dram_parameter("bo_pm", [128, MT], F32, isOutput=False)
    maskT = nc.declare_dram_parameter("maskT", [128, 128], BF16, isOutput=False)
    ident = nc.declare_dram_parameter("ident", [128, 128], BF16, isOutput=False)
    outT = nc.declare_dram_parameter("outT", [B, DIM, TOW], F32, isOutput=True)

    with tile.TileContext(nc) as tc:
        with (
            tc.tile_pool(name="wpool", bufs=1) as wpool,
            tc.tile_pool(name="xpool", bufs=2) as xpool,
            tc.tile_pool(name="actpool", bufs=2) as actpool,
            tc.tile_pool(name="spool", bufs=2) as spool,
            tc.tile_pool(name="opool", bufs=3) as opool,
            tc.tile_pool(name="pscores", bufs=2, space="PSUM") as pscores,
            tc.tile_pool(name="pattnT", bufs=1, space="PSUM") as pattnT,
            tc.tile_pool(name="ppv", bufs=1, space="PSUM") as ppv,
            tc.tile_pool(name="pproj", bufs=2, space="PSUM") as pproj,
        ):
            # ---- replicated constants (loaded once) ----
            wq_sb = wpool.tile([128, KT, DIM], BF16)
            wkv_sb = wpool.tile([128, KT, 2 * DIM], BF16)
            wo_sb = wpool.tile([128, KT, DIM], BF16)
            bo_sb = wpool.tile([128, MT], F32)
            mask_sb = wpool.tile([128, 128], BF16)
            id_sb = wpool.tile([128, 128], BF16)
            for k in range(KT):
                nc.sync.dma_start(out=wq_sb[:, k, :], in_=wq[k * 128:(k + 1) * 128, :])
                nc.sync.dma_start(out=wkv_sb[:, k, :], in_=wkv[k * 128:(k + 1) * 128, :])
                nc.sync.dma_start(out=wo_sb[:, k, :], in_=wo[k * 128:(k + 1) * 128, :])
            nc.sync.dma_start(out=bo_sb, in_=bo_pm[:])
            nc.sync.dma_start(out=mask_sb, in_=maskT[:])
            nc.sync.dma_start(out=id_sb, in_=ident[:])

            for step in [ss % NSTEP for ss in range(NSTEP * REPEAT)]:
                b, half = step // 2, step % 2
                toff = half * HB          # own-token offset within batch

                # ---- load this step's xT slice (halo + own tokens) ----
                xs = xpool.tile([128, KT, HTH], BF16, tag="x")
                nc.gpsimd.dma_start(
                    out=xs[:],
                    in_=xT[b].rearrange("(k p) t -> p k t", p=128)[
                        :, :, toff:toff + HTH
                    ],
                )

                qT = actpool.tile([128, MT, HB], BF16, tag="qT")
                kTt = actpool.tile([128, MT, HTH], BF16, tag="kT")
                v_sb = actpool.tile([128, NWH + 1, DIM], BF16, tag="v")
                aoT = actpool.tile([128, MT, HB], BF16, tag="aoT")

                # ---- q projection, feature-major: qT[m] = wq[:,m].T @ x ----
                for m in range(MT):
                    ps = pproj.tile([128, 512], F32, tag="proj")
                    for k in range(KT):
                        nc.tensor.matmul(
                            ps,
                            lhsT=wq_sb[:, k, m * 128:(m + 1) * 128],
                            rhs=xs[:, k, W:W + HB],
                            start=(k == 0),
                            stop=(k == KT - 1),
                        )
                    nc.vector.tensor_copy(out=qT[:, m, :], in_=ps)

                # ---- k projection, feature-major (incl. halo) ----
                for m in range(MT):
                    for c in range(2):
                        ps = pproj.tile([128, 512], F32, tag="proj")
                        cw = 320  # 2 x 320 = 640 columns
                        for k in range(KT):
                            nc.tensor.matmul(
                                ps[:, 0:cw],
                                lhsT=wkv_sb[:, k, m * 128:(m + 1) * 128],
                                rhs=xs[:, k, c * cw:(c + 1) * cw],
                                start=(k == 0),
                                stop=(k == KT - 1),
                            )
                        nc.vector.tensor_copy(
                            out=kTt[:, m, c * cw:(c + 1) * cw], in_=ps[:, 0:cw]
                        )

                # ---- v projection, token-major (incl. halo) ----
                for wi in range(NWH + 1):
                    for c in range(2):
                        ps = pproj.tile([128, 512], F32, tag="proj")
                        for k in range(KT):
                            nc.tensor.matmul(
                                ps,
                                lhsT=xs[:, k, wi * 128:(wi + 1) * 128],
                                rhs=wkv_sb[:, k, DIM + c * 512:DIM + (c + 1) * 512],
                                start=(k == 0),
                                stop=(k == KT - 1),
                            )
                        nc.vector.tensor_copy(
                            out=v_sb[:, wi, c * 512:(c + 1) * 512], in_=ps
                        )

                # ---- attention: 4 windows x 4 groups of 4 heads ----
                for w in range(NWH):
                    for g in range(4):
                        sc = pscores.tile([128, 4, 2 * W], F32, tag="scores")
                        for hh in range(4):
                            h = 4 * g + hh
                            m, r = h // 2, (h % 2) * 64
                            nc.tensor.matmul(
                                sc[:, hh, 0:2 * W],
                                lhsT=qT[r:r + 64, m, w * W:(w + 1) * W],
                                rhs=kTt[r:r + 64, m, w * W:w * W + 2 * W],
                                start=True,
                                stop=True,
                            )
                            # additive causal mask on the current-window half:
                            # sc[:, hh, W+jc] += maskT[jc, i]  (-1e30 where jc > i)
                            nc.tensor.matmul(
                                sc[:, hh, W:2 * W],
                                lhsT=mask_sb,
                                rhs=id_sb,
                                start=False,
                                stop=False,
                                skip_group_check=True,
                            )
                        # softmax (no max-subtraction; scores are O(1));
                        # exp + per-head row-sum fused on ACT via accum_out
                        exps = spool.tile([128, 4, 2 * W], F32, tag="expS")
                        sums = spool.tile([128, 4], F32, tag="sums")
                        for hh in range(4):
                            nc.scalar.activation(
                                out=exps[:, hh, :],
                                in_=sc[:, hh, 0:2 * W],
                                func=AF.Exp,
                                bias=0.0,
                                scale=SCALE,
                                accum_out=sums[:, hh:hh + 1],
                            )
                        recip = spool.tile([128, 4], F32, tag="recip")
                        nc.vector.reciprocal(out=recip, in_=sums)
                        attn = spool.tile([128, 4, 2 * W], BF16, tag="attnb")
                        for hh in range(4):
                            nc.scalar.activation(
                                out=attn[:, hh, :],
                                in_=exps[:, hh, :],
                                func=AF.Copy,
                                bias=0.0,
                                scale=recip[:, hh:hh + 1],
                            )
                        # transpose probs on PE: [i, j] -> [j, i]
                        pT = pattnT.tile([128, 8, 128], BF16, tag="attnT")
                        for hh in range(4):
                            for hf in range(2):
                                nc.tensor.transpose(
                                    out=pT[:, hh * 2 + hf, :],
                                    in_=attn[:, hh, hf * W:(hf + 1) * W],
                                    identity=id_sb,
                                )
                        aT = spool.tile([128, 8, 128], BF16, tag="attnT_sb")
                        nc.vector.tensor_copy(out=aT, in_=pT)
                        # pv: outT[dh, i] = v[j, dh].T @ attnT[j, i]
                        pv = ppv.tile([128, 2, 128], F32, tag="pv")
                        for hh in range(4):
                            h = 4 * g + hh
                            pr, pc = (hh % 2) * 64, hh // 2
                            for hf in range(2):
                                nc.tensor.matmul(
                                    pv[pr:pr + 64, pc, :],
                                    lhsT=v_sb[:, w + hf, h * 64:(h + 1) * 64],
                                    rhs=aT[:, hh * 2 + hf, :],
                                    start=(hf == 0),
                                    stop=(hf == 1),
                                )
                        for pc in range(2):
                            nc.vector.tensor_copy(
                                out=aoT[:, 2 * g + pc, w * W:(w + 1) * W],
                                in_=pv[:, pc, :],
                            )

                # ---- output projection + bias, feature-major ----
                for m in range(MT):
                    ps = pproj.tile([128, 512], F32, tag="proj")
                    for k in range(KT):
                        nc.tensor.matmul(
                            ps,
                            lhsT=wo_sb[:, k, m * 128:(m + 1) * 128],
                            rhs=aoT[:, k, :],
                            start=(k == 0),
                            stop=(k == KT - 1),
                        )
                    osb = opool.tile([128, 512], F32, tag="outsb")
                    nc.vector.tensor_scalar_add(
                        out=osb, in0=ps, scalar1=bo_sb[:, m:m + 1]
                    )
                    nc.sync.dma_start(
                        out=outT[b, m * 128:(m + 1) * 128, toff:toff + HB],
                        in_=osb,
                    )
    nc.compile()
    return nc


_NC_CACHE = None


def _get_nc():
    global _NC_CACHE
    if _NC_CACHE is None:
        _NC_CACHE = _build_bass()
    return _NC_CACHE


def kernel(x, wq, wkv, wo, bo):
    global LAST_RESULT
    bfd = ml_dtypes.bfloat16
    x = np.asarray(x, np.float32)
    wq_b = np.asarray(wq, np.float32).astype(bfd)
    wkv_b = np.asarray(wkv, np.float32).astype(bfd)
    wo_b = np.asarray(wo, np.float32).astype(bfd)
    bo_pm = np.ascontiguousarray(
        np.asarray(bo, np.float32).reshape(MT, 128).T
    )
    # maskT[jc, i] = -1e30 where current-window col jc > row i (causal)
    maskT = np.where(
        np.arange(W)[:, None] > np.arange(W)[None, :], -1e30, 0.0
    ).astype(bfd)
    ident = np.eye(128, dtype=bfd)

    xb = x.astype(bfd)
    in_maps = []
    for c in range(NCORES):
        lo, hi = c * TOW - W, (c + 1) * TOW
        if c == 0:
            sl = np.concatenate(
                [np.zeros((B, W, DIM), bfd), xb[:, :hi]], axis=1
            )
        else:
            sl = xb[:, lo:hi]
        xT_c = np.ascontiguousarray(sl.transpose(0, 2, 1))  # [B, DIM, TH]
        in_maps.append(
            dict(xT=xT_c, wq=wq_b, wkv=wkv_b, wo=wo_b, bo_pm=bo_pm,
                 maskT=maskT, ident=ident)
        )

    nc = _get_nc()
    res = run_bass_kernel_spmd(
        nc, in_maps, list(range(NCORES)), trace=TRACE, **TRACE_KW
    )
    if TRACE:
        LAST_RESULT = res
    out = np.empty((B, N, DIM), np.float32)
    for c in range(NCORES):
        out[:, c * TOW:(c + 1) * TOW, :] = res.results[c]["outT"].transpose(0, 2, 1)
    return out


# revision 10
# speedup vs baseline: 1.3465x; 1.2822x over previous
"""Local (windowed) attention kernel for TRN2, 8 NeuronCores, SPMD.

Reference computation (B=4, N=8192, DIM=1024, H=16, DH=64, W=128):
    q = x @ wq ; k,v = split(x @ wkv)
    per (batch, head, window of 128): attend to [prev window, cur window]
    with causal mask (j > i + W masked), softmax, out = attn @ v
    out = out @ wo + bo

Sharding: sequence dim split into 8 contiguous chunks of 1024 tokens, one
per core.  Each core receives its x slice with a 128-token halo in front
(zeros for core 0 — matches the reference's zero-pad of the first window)
and computes q/k/v projections, attention, and the output projection for
its own tokens only.  Weights are replicated.  No collectives.

Processing is software-pipelined at half-batch granularity (512 own
tokens + 128-token halo per step): all activation tiles are
double-buffered so the Tile scheduler can overlap step s+1's dense
projection matmuls with step s's attention (whose PE work is interleaved
with ACT softmax / DVE copies).  This keeps the PE free of multi-us idle
gaps so the HAM clock gate stays at full rate.

On-chip layout is feature-major (x fed pre-transposed as [dim, token]),
so every matmul uses natural HBM weight layouts and no on-chip
transposes are needed except the attention-probs transpose (done on PE).
All matmuls run in bf16 (1 cycle/row on the PE) with fp32 PSUM
accumulation; softmax runs in fp32.  exp() is computed without
max-subtraction: scores are O(1) here (|s| < ~4), so overflow is
impossible and this matches jax.nn.softmax to fp32 roundoff.
"""

import numpy as np
import ml_dtypes

import concourse.bass as bass
import concourse.bacc as bacc
import concourse.mybir as mybir
import concourse.tile as tile
from concourse.bass_utils import run_bass_kernel_spmd

B, N, DIM = 4, 8192, 1024
H, DH, W = 16, 64, 128
NCORES = 8
TOW = N // NCORES          # own tokens per core per batch   = 1024
TH = TOW + W               # with front halo                 = 1152
HB = TOW // 2              # own tokens per half-batch step  = 512
HTH = HB + W               # step tokens with halo           = 640
NWH = HB // W              # windows per step                = 4
NSTEP = 2 * B              # pipeline steps per core         = 8
KT = DIM // 128            # contraction tiles               = 8
MT = DIM // 128            # inner/output tiles              = 8
SCALE = DH ** -0.5

BF16 = mybir.dt.bfloat16
F32 = mybir.dt.float32
AX = mybir.AxisListType
AF = mybir.ActivationFunctionType

TRACE = False              # set by test.py to collect an NTFF profile
TRACE_KW = {}
LAST_RESULT = None         # BassKernelResults stash when TRACE
REPEAT = 1                 # whole-computation repeats inside the NEFF (bench)


def _build_bass():
    nc = bacc.Bacc(None, target_bir_lowering=False)
    xT = nc.declare_dram_parameter("xT", [B, DIM, TH], BF16, isOutput=False)
    wq = nc.declare_dram_parameter("wq", [DIM, DIM], BF16, isOutput=False)
    wkv = nc.declare_dram_parameter("wkv", [DIM, 2 * DIM], BF16, isOutput=False)
    wo = nc.declare_dram_parameter("wo", [DIM, DIM], BF16, isOutput=False)
    bo_pm = nc.declare_dram_parameter("bo_pm", [128, MT], F32, isOutput=False)
    maskT = nc.declare_dram_parameter("maskT", [128, 128], BF16, isOutput=False)
    ident = nc.declare_dram_parameter("ident", [128, 128], BF16, isOutput=False)
    outT = nc.declare_dram_parameter("outT", [B, DIM, TOW], F32, isOutput=True)

    with tile.TileContext(nc) as tc:
        with (
            tc.tile_pool(name="wpool", bufs=1) as wpool,
            tc.tile_pool(name="xpool", bufs=2) as xpool,
            tc.tile_pool(name="actpool", bufs=2) as actpool,
            tc.tile_pool(name="spool", bufs=2) as spool,
            tc.tile_pool(name="opool", bufs=3) as opool,
            tc.tile_pool(name="pscores", bufs=2, space="PSUM") as pscores,
            tc.tile_pool(name="pattnT", bufs=1, space="PSUM") as pattnT,
            tc.tile_pool(name="ppv", bufs=1, space="PSUM") as ppv,
            tc.tile_pool(name="pproj", bufs=2, space="PSUM") as pproj,
        ):
            # ---- replicated constants (loaded once) ----
            wq_sb = wpool.tile([128, KT, DIM], BF16)
            wkv_sb = wpool.tile([128, KT, 2 * DIM], BF16)
            wo_sb = wpool.tile([128, KT, DIM], BF16)
            bo_sb = wpool.tile([128, MT], F32)
            mask_sb = wpool.tile([128, 128], BF16)
            id_sb = wpool.tile([128, 128], BF16)
            for k in range(KT):
                nc.sync.dma_start(out=wq_sb[:, k, :], in_=wq[k * 128:(k + 1) * 128, :])
                nc.sync.dma_start(out=wkv_sb[:, k, :], in_=wkv[k * 128:(k + 1) * 128, :])
                nc.sync.dma_start(out=wo_sb[:, k, :], in_=wo[k * 128:(k + 1) * 128, :])
            nc.sync.dma_start(out=bo_sb, in_=bo_pm[:])
            nc.sync.dma_start(out=mask_sb, in_=maskT[:])
            nc.sync.dma_start(out=id_sb, in_=ident[:])

            steps = [ss % NSTEP for ss in range(NSTEP * REPEAT)]
            xs_t = [None] * NSTEP
            qT_t = [None] * NSTEP
            kT_t = [None] * NSTEP
            v_t = [None] * NSTEP
            ao_t = [None] * NSTEP

            def emit_xload(s):
                b, half = s // 2, s % 2
                toff = half * HB
                xs = xpool.tile([128, KT, HTH], BF16, tag="x")
                nc.gpsimd.dma_start(
                    out=xs[:],
                    in_=xT[b].rearrange("(k p) t -> p k t", p=128)[
                        :, :, toff:toff + HTH
                    ],
                )
                xs_t[s] = xs

            def proj_units(s):
                xs = xs_t[s]
                qT = actpool.tile([128, MT, HB], BF16, tag="qT")
                kTt = actpool.tile([128, MT, HTH], BF16, tag="kT")
                v_sb = actpool.tile([128, NWH + 1, DIM], BF16, tag="v")
                aoT = actpool.tile([128, MT, HB], BF16, tag="aoT")
                qT_t[s], kT_t[s], v_t[s], ao_t[s] = qT, kTt, v_sb, aoT

                for m in range(MT):
                    ps = pproj.tile([128, 512], F32, tag="proj")
                    for k in range(KT):
                        nc.tensor.matmul(
                            ps,
                            lhsT=wq_sb[:, k, m * 128:(m + 1) * 128],
                            rhs=xs[:, k, W:W + HB],
                            start=(k == 0),
                            stop=(k == KT - 1),
                        )
                    nc.vector.tensor_copy(out=qT[:, m, :], in_=ps)
                    yield

                cw = 320
                for m in range(MT):
                    for c in range(2):
                        ps = pproj.tile([128, 512], F32, tag="proj")
                        for k in range(KT):
                            nc.tensor.matmul(
                                ps[:, 0:cw],
                                lhsT=wkv_sb[:, k, m * 128:(m + 1) * 128],
                                rhs=xs[:, k, c * cw:(c + 1) * cw],
                                start=(k == 0),
                                stop=(k == KT - 1),
                            )
                        nc.vector.tensor_copy(
                            out=kTt[:, m, c * cw:(c + 1) * cw], in_=ps[:, 0:cw]
                        )
                        yield

                for wi in range(NWH + 1):
                    for c in range(2):
                        ps = pproj.tile([128, 512], F32, tag="proj")
                        for k in range(KT):
                            nc.tensor.matmul(
                                ps,
                                lhsT=xs[:, k, wi * 128:(wi + 1) * 128],
                                rhs=wkv_sb[:, k, DIM + c * 512:DIM + (c + 1) * 512],
                                start=(k == 0),
                                stop=(k == KT - 1),
                            )
                        nc.vector.tensor_copy(
                            out=v_sb[:, wi, c * 512:(c + 1) * 512], in_=ps
                        )
                        yield

            def emit_attn_group(s, w, g):
                qT, kTt, v_sb, aoT = qT_t[s], kT_t[s], v_t[s], ao_t[s]
                sc = pscores.tile([128, 4, 2 * W], F32, tag="scores")
                for hh in range(4):
                    h = 4 * g + hh
                    m, r = h // 2, (h % 2) * 64
                    nc.tensor.matmul(
                        sc[:, hh, 0:2 * W],
                        lhsT=qT[r:r + 64, m, w * W:(w + 1) * W],
                        rhs=kTt[r:r + 64, m, w * W:w * W + 2 * W],
                        start=True,
                        stop=True,
                    )
                    nc.tensor.matmul(
                        sc[:, hh, W:2 * W],
                        lhsT=mask_sb,
                        rhs=id_sb,
                        start=False,
                        stop=False,
                        skip_group_check=True,
                    )
                exps = spool.tile([128, 4, 2 * W], F32, tag="expS")
                sums = spool.tile([128, 4], F32, tag="sums")
                for hh in range(4):
                    nc.scalar.activation(
                        out=exps[:, hh, :],
                        in_=sc[:, hh, 0:2 * W],
                        func=AF.Exp,
                        bias=0.0,
                        scale=SCALE,
                        accum_out=sums[:, hh:hh + 1],
                    )
                recip = spool.tile([128, 4], F32, tag="recip")
                nc.vector.reciprocal(out=recip, in_=sums)
                attn = spool.tile([128, 4, 2 * W], BF16, tag="attnb")
                for hh in range(4):
                    nc.scalar.activation(
                        out=attn[:, hh, :],
                        in_=exps[:, hh, :],
                        func=AF.Copy,
                        bias=0.0,
                        scale=recip[:, hh:hh + 1],
                    )
                pT = pattnT.tile([128, 8, 128], BF16, tag="attnT")
                for hh in range(4):
                    for hf in range(2):
                        nc.tensor.transpose(
                            out=pT[:, hh * 2 + hf, :],
                            in_=attn[:, hh, hf * W:(hf + 1) * W],
                            identity=id_sb,
                        )
                aT = spool.tile([128, 8, 128], BF16, tag="attnT_sb")
                nc.vector.tensor_copy(out=aT, in_=pT)
                pv = ppv.tile([128, 2, 128], F32, tag="pv")
                for hh in range(4):
                    h = 4 * g + hh
                    pr, pc = (hh % 2) * 64, hh // 2
                    for hf in range(2):
                        nc.tensor.matmul(
                            pv[pr:pr + 64, pc, :],
                            lhsT=v_sb[:, w + hf, h * 64:(h + 1) * 64],
                            rhs=aT[:, hh * 2 + hf, :],
                            start=(hf == 0),
                            stop=(hf == 1),
                        )
                for pc in range(2):
                    nc.vector.tensor_copy(
                        out=aoT[:, 2 * g + pc, w * W:(w + 1) * W],
                        in_=pv[:, pc, :],
                    )

            def emit_outproj(s):
                b, half = s // 2, s % 2
                toff = half * HB
                aoT = ao_t[s]
                for m in range(MT):
                    ps = pproj.tile([128, 512], F32, tag="proj")
                    for k in range(KT):
                        nc.tensor.matmul(
                            ps,
                            lhsT=wo_sb[:, k, m * 128:(m + 1) * 128],
                            rhs=aoT[:, k, :],
                            start=(k == 0),
                            stop=(k == KT - 1),
                        )
                    osb = opool.tile([128, 512], F32, tag="outsb")
                    nc.vector.tensor_scalar_add(
                        out=osb, in0=ps, scalar1=bo_sb[:, m:m + 1]
                    )
                    nc.sync.dma_start(
                        out=outT[b, m * 128:(m + 1) * 128, toff:toff + HB],
                        in_=osb,
                    )

            emit_xload(steps[0])
            gen = proj_units(steps[0])
            for _ in gen:
                pass
            for si, s in enumerate(steps):
                nxt = steps[si + 1] if si + 1 < len(steps) else None
                if nxt is not None:
                    emit_xload(nxt)
                    gen = proj_units(nxt)
                else:
                    gen = iter(())
                for gi in range(NWH * 4):
                    emit_attn_group(s, gi // 4, gi % 4)
                    next(gen, None)
                    if gi % 2 == 0:
                        next(gen, None)
                for _ in gen:
                    pass
                emit_outproj(s)
    nc.compile()
    return nc


_NC_CACHE = None


def _get_nc():
    global _NC_CACHE
    if _NC_CACHE is None:
        _NC_CACHE = _build_bass()
    return _NC_CACHE


def kernel(x, wq, wkv, wo, bo):
    global LAST_RESULT
    bfd = ml_dtypes.bfloat16
    x = np.asarray(x, np.float32)
    wq_b = np.asarray(wq, np.float32).astype(bfd)
    wkv_b = np.asarray(wkv, np.float32).astype(bfd)
    wo_b = np.asarray(wo, np.float32).astype(bfd)
    bo_pm = np.ascontiguousarray(
        np.asarray(bo, np.float32).reshape(MT, 128).T
    )
    # maskT[jc, i] = -1e30 where current-window col jc > row i (causal)
    maskT = np.where(
        np.arange(W)[:, None] > np.arange(W)[None, :], -1e30, 0.0
    ).astype(bfd)
    ident = np.eye(128, dtype=bfd)

    xb = x.astype(bfd)
    in_maps = []
    for c in range(NCORES):
        lo, hi = c * TOW - W, (c + 1) * TOW
        if c == 0:
            sl = np.concatenate(
                [np.zeros((B, W, DIM), bfd), xb[:, :hi]], axis=1
            )
        else:
            sl = xb[:, lo:hi]
        xT_c = np.ascontiguousarray(sl.transpose(0, 2, 1))  # [B, DIM, TH]
        in_maps.append(
            dict(xT=xT_c, wq=wq_b, wkv=wkv_b, wo=wo_b, bo_pm=bo_pm,
                 maskT=maskT, ident=ident)
        )

    nc = _get_nc()
    res = run_bass_kernel_spmd(
        nc, in_maps, list(range(NCORES)), trace=TRACE, **TRACE_KW
    )
    if TRACE:
        LAST_RESULT = res
    out = np.empty((B, N, DIM), np.float32)
    for c in range(NCORES):
        out[:, c * TOW:(c + 1) * TOW, :] = res.results[c]["outT"].transpose(0, 2, 1)
    return out
